# revision 1
# baseline (speedup 1.0000x reference)
"""Dice-coefficient-mean kernel for Trainium2 (8 NeuronCores, SPMD).

Sharding: data-parallel over batch — core b processes batch b
(128^3 = 2,097,216.. = 2,097,152 elements per tensor, laid out [128, 16384]).

Per core we need, per label l:
  c1[l] = #(s1 == l), c2[l] = #(s2 == l), inter[l] = #(s1 == l & s2 == l)
recovered exactly from 40 scalar statistics:
  pair = 16*s1 + s2 (fp16, exact)
  inter[l]  = #(pair == 17l)        l = 0..13
  F1[T]     = #(pair <= 16T+15)     T = 0..12   (cumulative counts of s1)
  F2[T]     = #(s2 <= T)            T = 0..12   (cumulative counts of s2)

Engine assignment (measured HW rates):
  - PE-path stats: DVE writes a fp16 0/1 mask (fast, no accum), PE reduces it
    with a ones-vector matmul chain into PSUM [1,512], DVE tensor_reduce ->
    scalar. ~6-7us/stat, pipelined across DVE/PE.
  - ACT-path stats: ScalarE Sign(x - (T+0.5)) with free-dim accumulator
    (#gt - #le per partition). ~11us/stat.
DVE accum_out is avoided entirely (measured pathologically slow on HW).
All statistics are exact integer counts; the final tiny reduction and dice
math run on host in float64.
"""

import numpy as np

NUM_LABELS = 14
EPS = float(np.finfo(float).eps)
B = 8
P = 128
FREE = 16384  # 128*128*128 / 128
NCHUNK = 4
K_ACT = 15  # stats assigned to the ScalarE sign path (of 26 cumulative stats)
MM_N = 512  # matmul moving free dim

_CACHE = {}


def _stat_specs(k_act):
    """Returns (pe_spec, act_spec).

    pe_spec entries: (src, op, const) with src in {pair, s1, s2},
        op in {eq, le}; counted via DVE mask + PE reduce.
    act_spec entries: (src, thr): #(src <= thr-0.5 ... ) via Sign.
    """
    act = []
    pe = [("pair", "eq", 17.0 * l) for l in range(NUM_LABELS)]
    # c2 cumulative: prefer ACT; c1 cumulative: fill ACT remainder, rest PE
    n_act_c2 = min(k_act, NUM_LABELS - 1)
    n_act_c1 = min(max(k_act - n_act_c2, 0), NUM_LABELS - 1)
    for t in range(NUM_LABELS - 1):
        if t < n_act_c2:
            act.append(("s2", t + 0.5))
        else:
            pe.append(("s2", "le", float(t)))
    for t in range(NUM_LABELS - 1):
        if t < n_act_c1:
            act.append(("s1", t + 0.5))
        else:
            pe.append(("pair", "le", 16.0 * t + 15.0))
    return pe, act


def _build(k_act=K_ACT, nchunk=NCHUNK, repeat=1, io_only=False, full_repeat=False):
    from concourse import bacc, mybir, tile

    pe_spec, act_spec = _stat_specs(k_act)
    npe, na = len(pe_spec), len(act_spec)

    nc = bacc.Bacc("TRN2", target_bir_lowering=False)
    s1 = nc.dram_tensor("s1", [P, FREE], mybir.dt.float32, kind="ExternalInput")
    s2 = nc.dram_tensor("s2", [P, FREE], mybir.dt.float32, kind="ExternalInput")
    out_p = nc.dram_tensor(
        "stats_pe", [1, max(nchunk * npe, 1)], mybir.dt.float32,
        kind="ExternalOutput",
    )
    out_a = nc.dram_tensor(
        "stats_act", [P, max(nchunk * na, 1)], mybir.dt.float32,
        kind="ExternalOutput",
    )

    cf = FREE // nchunk
    nmm = cf // MM_N
    op = mybir.AluOpType
    with tile.TileContext(nc) as tc:
        data_bufs = 3 if nchunk >= 4 else 2
        with (
            tc.tile_pool(name="data", bufs=data_bufs) as pool,
            tc.tile_pool(name="mask", bufs=data_bufs) as maskp,
            tc.tile_pool(name="aux", bufs=1) as aux,
            tc.tile_pool(name="psum", bufs=6, space="PSUM") as psum,
        ):
            stats_p = aux.tile([1, max(nchunk * npe, 1)], mybir.dt.float32)
            stats_a = aux.tile([P, max(nchunk * na, 1)], mybir.dt.float32)
            junk_a = aux.tile([P, cf], mybir.dt.float16)
            ones = aux.tile([P, 1], mybir.dt.float16)
            nc.vector.memset(ones[:], 1.0)
            if io_only:
                nc.vector.memset(stats_p[:], 0.0)
                nc.vector.memset(stats_a[:], 0.0)
            if na and not io_only:
                biases = aux.tile([P, na], mybir.dt.float32)
                for i, (_, thr) in enumerate(act_spec):
                    nc.vector.memset(biases[:, i : i + 1], -thr)
            outer = repeat if full_repeat else 1
            inner = 1 if full_repeat else repeat
            for rr in range(outer):
              for c in range(nchunk):
                s1h = pool.tile([P, cf], mybir.dt.float16, tag="s1h")
                s2h = pool.tile([P, cf], mybir.dt.float16, tag="s2h")
                # SWDGE casts f32 -> fp16 inline with the HBM load
                nc.gpsimd.dma_start(out=s1h[:], in_=s1[:, c * cf : (c + 1) * cf])
                nc.gpsimd.dma_start(out=s2h[:], in_=s2[:, c * cf : (c + 1) * cf])
                if io_only:
                    continue
                pair = pool.tile([P, cf], mybir.dt.float16, tag="pair")
                nc.vector.scalar_tensor_tensor(
                    out=pair[:], in0=s1h[:], scalar=16.0, in1=s2h[:],
                    op0=op.mult, op1=op.add,
                )
                for _r in range(inner):
                    srcs = {"pair": pair, "s1": s1h, "s2": s2h}
                    for i, (src, kind, const) in enumerate(pe_spec):
                        mask = maskp.tile([P, cf], mybir.dt.float16, tag="mask")
                        nc.vector.tensor_scalar(
                            out=mask[:], in0=srcs[src][:], scalar1=const,
                            scalar2=None,
                            op0=op.is_equal if kind == "eq" else op.is_le,
                        )
                        acc = psum.tile([1, MM_N], mybir.dt.float32, tag="acc")
                        for kk in range(nmm):
                            nc.tensor.matmul(
                                acc[:],
                                ones[:],
                                mask[:, kk * MM_N : (kk + 1) * MM_N],
                                start=(kk == 0),
                                stop=(kk == nmm - 1),
                            )
                        nc.vector.tensor_reduce(
                            out=stats_p[:1, c * npe + i : c * npe + i + 1],
                            in_=acc[:1, :],
                            axis=mybir.AxisListType.X,
                            op=op.add,
                        )
                    for i, (src, thr) in enumerate(act_spec):
                        nc.scalar.activation(
                            out=junk_a[:], in_=srcs[src][:],
                            func=mybir.ActivationFunctionType.Sign,
                            bias=biases[:, i : i + 1], scale=1.0,
                            accum_out=stats_a[:, c * na + i : c * na + i + 1],
                        )
            nc.sync.dma_start(out=out_p[:], in_=stats_p[:])
            nc.sync.dma_start(out=out_a[:], in_=stats_a[:])
    nc.compile()
    return nc, pe_spec, act_spec, npe, na


def _get_built(k_act=K_ACT, nchunk=NCHUNK, repeat=1, io_only=False,
               full_repeat=False):
    key = (k_act, nchunk, repeat, io_only, full_repeat)
    if key not in _CACHE:
        _CACHE[key] = _build(k_act, nchunk, repeat, io_only, full_repeat)
    return _CACHE[key]


LAST_EXEC_NS = None
LAST_RESULTS = None


def _decode(results, pe_spec, act_spec, npe, na, nchunk):
    cf = FREE // nchunk
    n_total = float(P * FREE)
    dice = np.zeros((B, NUM_LABELS), dtype=np.float64)
    for b in range(B):
        sp = np.asarray(results[b]["stats_pe"], dtype=np.float64)
        sp = sp.reshape(nchunk, npe).sum(axis=0) if npe else sp
        if na:
            sa = np.asarray(results[b]["stats_act"], dtype=np.float64)
            sa = sa.reshape(P, nchunk, na)
            # sign stat S = #gt - #le  ->  #le = (cf - S)/2 per (partition,chunk)
            sa = ((cf - sa) / 2.0).sum(axis=(0, 1))  # [na]
        inter = np.zeros(NUM_LABELS)
        f1 = np.zeros(NUM_LABELS)
        f2 = np.zeros(NUM_LABELS)
        for i, (src, kind, const) in enumerate(pe_spec):
            v = sp[i]
            if src == "pair" and kind == "eq":
                inter[int(round(const / 17.0))] = v
            elif src == "pair" and kind == "le":
                f1[int(round((const - 15.0) / 16.0))] = v
            else:  # s2 le
                f2[int(round(const))] = v
        for i, (src, thr) in enumerate(act_spec):
            t = int(round(thr - 0.5))
            if src == "s1":
                f1[t] = sa[i]
            else:
                f2[t] = sa[i]
        f1[NUM_LABELS - 1] = n_total
        f2[NUM_LABELS - 1] = n_total
        c1 = np.diff(f1, prepend=0.0)
        c2 = np.diff(f2, prepend=0.0)
        dice[b] = 2.0 * inter / (c1 + c2 + EPS)
    resv = dice.reshape(-1)
    total = resv.sum()
    nz = float((resv > 0).sum())
    mean = total / nz if nz > 0 else 0.0
    return np.float32(mean)


def _run(segment1, segment2, trace=False, k_act=K_ACT, nchunk=NCHUNK):
    global LAST_EXEC_NS, LAST_RESULTS
    from concourse.bass_utils import run_bass_kernel_spmd

    nc, pe_spec, act_spec, npe, na = _get_built(k_act, nchunk)

    seg1 = np.ascontiguousarray(np.asarray(segment1, dtype=np.float32)).reshape(
        B, P, FREE
    )
    seg2 = np.ascontiguousarray(np.asarray(segment2, dtype=np.float32)).reshape(
        B, P, FREE
    )
    in_maps = [{"s1": seg1[b], "s2": seg2[b]} for b in range(B)]
    res = run_bass_kernel_spmd(nc, in_maps, core_ids=list(range(B)), trace=trace)
    LAST_EXEC_NS = res.exec_time_ns
    LAST_RESULTS = res
    return _decode(res.results, pe_spec, act_spec, npe, na, nchunk)


def kernel(segment1, segment2):
    return _run(segment1, segment2, trace=False)


def benchmark(segment1, segment2):
    """Run with NTFF tracing; returns exec_time_ns (may be None if no hook)."""
    _run(segment1, segment2, trace=True)
    return LAST_EXEC_NS

